# revision 1
# baseline (speedup 1.0000x reference)
import numpy as np

B, S, H, V = 4, 512, 768, 32


def _valid_indices():
    idx = np.arange(S)[:, None] + np.arange(V)[None, :]  # [S, V]
    mask = idx < S
    si, ji = np.nonzero(mask)
    return idx, si, ji


def _compute_np(seq_hiddens, W, b):
    seq_hiddens = np.asarray(seq_hiddens, dtype=np.float32)
    W = np.asarray(W, dtype=np.float32)
    b = np.asarray(b, dtype=np.float32)
    idx, si, ji = _valid_indices()
    padded = np.pad(seq_hiddens, ((0, 0), (0, V - 1), (0, 0)))
    visual = padded[:, idx, :]                              # [B, S, V, H]
    denom = np.arange(1, V + 1, dtype=np.float32)[None, None, :, None]
    context = np.cumsum(visual, axis=2, dtype=np.float32) / denom
    W1, W2, W3 = W[:, :H], W[:, H:2 * H], W[:, 2 * H:]
    rep_proj = seq_hiddens @ W1.T                           # [B, S, H]
    vis_proj = visual.reshape(-1, H) @ W2.T
    ctx_proj = context.reshape(-1, H) @ W3.T
    out = np.tanh(rep_proj[:, :, None, :]
                  + vis_proj.reshape(B, S, V, H)
                  + ctx_proj.reshape(B, S, V, H)
                  + b)
    return np.ascontiguousarray(out[:, si, ji, :].astype(np.float32))


def kernel(seq_hiddens, W, b):
    # Data-parallel over the 8 NeuronCores when JAX can see them; the
    # batch (B=4) is smaller than the core count, so shard B*S rows.
    try:
        import jax
        import jax.numpy as jnp
        devs = jax.devices()
        n = min(8, len(devs))
        if n >= 2:
            sh = np.asarray(seq_hiddens, dtype=np.float32)
            Wn = np.asarray(W, dtype=np.float32)
            bn = np.asarray(b, dtype=np.float32)
            idx, si, ji = _valid_indices()
            padded = np.pad(sh, ((0, 0), (0, V - 1), (0, 0)))
            visual = padded[:, idx, :]                      # [B, S, V, H]
            rows = visual.reshape(B * S, V, H)
            reps = sh.reshape(B * S, H)
            per = (B * S) // n                              # 2048/8 = 256
            rows_sh = rows.reshape(n, per, V, H)
            reps_sh = reps.reshape(n, per, H)
            W1, W2, W3 = Wn[:, :H], Wn[:, H:2 * H], Wn[:, 2 * H:]
            denom = np.arange(1, V + 1, dtype=np.float32)[None, :, None]

            def shard_fn(vis, rep, w1t, w2t, w3t, bias, dn):
                ctx = jnp.cumsum(vis, axis=1) / dn
                rp = rep @ w1t                              # [per, H]
                vp = jnp.einsum('pvh,ho->pvo', vis, w2t)
                cp = jnp.einsum('pvh,ho->pvo', ctx, w3t)
                return jnp.tanh(rp[:, None, :] + vp + cp + bias)

            pf = jax.pmap(shard_fn, in_axes=(0, 0, None, None, None, None, None),
                          devices=devs[:n])
            out_sh = pf(rows_sh, reps_sh, W1.T, W2.T, W3.T, bn, denom)
            out = np.asarray(out_sh).reshape(B, S, V, H)
            return np.ascontiguousarray(out[:, si, ji, :].astype(np.float32))
    except Exception:
        pass
    return _compute_np(seq_hiddens, W, b)



# revision 3
# speedup vs baseline: 110300.9397x; 110300.9397x over previous
"""Handshaking kernel on 8 Trainium2 NeuronCores via Bass/Tile.

Math (per batch b, start s, window offset j < 32, feature o):
  out[b, s, j, o] = tanh( p1[b,s,o] + p2[b,s+j,o]
                          + (1/(j+1)) * sum_{u=s}^{s+j} p3[b,u,o] + bias[o] )
with pk[t] = x[t] @ Wk.T,  Wk = W[:, k*768:(k+1)*768]  (W indexed [o, h]).

Sharding: 8 cores, each takes 256 consecutive starts of one batch element
(core = 2*b + half).  The windowed terms only need a 31-row forward halo,
so every core gets an independent [287, 768] slice of x -- no collectives.

On-core layout is feature-major ([o partitions, t free]) so the window
shifts are free strided views; the per-(s,j) output rows are produced by
PE "scaled transpose" matmuls accumulating  P.T @ I + A.T @ (I/(j+1))
into PSUM, which ScalarE evacuates with fused tanh into natural layout
for contiguous 3KB-per-row DMA writes.
"""

import numpy as np

B, S, H, V = 4, 512, 768, 32
SC = 256          # starts per core
T = SC + V - 1    # 287: halo'd positions per core
NK = H // 128     # 6 feature tiles

_CACHE = {}


def _build_program():
    import concourse.bacc as bacc
    import concourse.bass as bass
    import concourse.mybir as mybir
    import concourse.tile as tile

    f32 = mybir.dt.float32
    bf16 = mybir.dt.bfloat16

    nc = bacc.Bacc("TRN2", target_bir_lowering=False, debug=False,
                   enable_asserts=False, num_devices=8)

    xT_d = nc.dram_tensor("xT", [NK * 128, T], bf16, kind="ExternalInput")
    wT_d = nc.dram_tensor("wT", [3 * NK * 128, H], bf16, kind="ExternalInput")
    idn_d = nc.dram_tensor("idn", [V * 128, 128], bf16, kind="ExternalInput")
    bv_d = nc.dram_tensor("bv", [NK * 128, 1], f32, kind="ExternalInput")
    out_d = nc.dram_tensor("out", [SC, V * H], f32, kind="ExternalOutput")

    with tile.TileContext(nc) as tc:
        with tc.tile_pool(name="persist", bufs=1) as persist:
            xt = persist.tile([128, NK * T], bf16, tag="xt")
            wt = persist.tile([128, 3 * NK * H], bf16, tag="wt")
            idn = persist.tile([128, V * 128], bf16, tag="idn")
            bv = persist.tile([128, NK], f32, tag="bv")
            p1b = persist.tile([128, NK * SC], bf16, tag="p1b")
            p2s = persist.tile([128, NK * T], bf16, tag="p2s")
            p3s = persist.tile([128, NK * T], bf16, tag="p3s")

            for k in range(NK):
                nc.sync.dma_start(xt[:, k * T:(k + 1) * T],
                                  xT_d[k * 128:(k + 1) * 128, :])
            for m in range(3 * NK):
                nc.sync.dma_start(wt[:, m * H:(m + 1) * H],
                                  wT_d[m * 128:(m + 1) * 128, :])
            for j in range(V):
                nc.sync.dma_start(idn[:, j * 128:(j + 1) * 128],
                                  idn_d[j * 128:(j + 1) * 128, :])
            for k in range(NK):
                nc.sync.dma_start(bv[:, k:k + 1], bv_d[k * 128:(k + 1) * 128, :])

            # ---- projections: p[o_tile i, t] = sum_k W[.,i].T @ x[k] ----
            with tc.tile_pool(name="ppsum", bufs=4, space="PSUM") as ppsum:
                for p in range(3):
                    for i in range(NK):
                        ps = ppsum.tile([128, T], f32, tag="ps")
                        for k in range(NK):
                            wcol = wt[:, (p * NK + k) * H + i * 128:
                                      (p * NK + k) * H + (i + 1) * 128]
                            nc.tensor.matmul(ps[:, :], wcol,
                                             xt[:, k * T:(k + 1) * T],
                                             start=(k == 0), stop=(k == NK - 1))
                        if p == 0:
                            nc.vector.tensor_scalar_add(
                                p1b[:, i * SC:(i + 1) * SC],
                                ps[:, 0:SC], bv[:, i:i + 1])
                        elif p == 1:
                            nc.scalar.copy(p2s[:, i * T:(i + 1) * T], ps[:, :])
                        else:
                            nc.scalar.copy(p3s[:, i * T:(i + 1) * T], ps[:, :])

            # ---- j loop ----
            with tc.tile_pool(name="jw", bufs=16) as jw, \
                 tc.tile_pool(name="jpsum", bufs=3, space="PSUM") as jpsum, \
                 tc.tile_pool(name="stage", bufs=4) as stpool:
                a_cur = [p3s[:, k * T: k * T + SC] for k in range(NK)]
                for j in range(V):
                    if j > 0:
                        a_new = []
                        for k in range(NK):
                            at = jw.tile([128, SC], bf16, tag="A")
                            nc.vector.tensor_add(
                                at[:, :], a_cur[k],
                                p3s[:, k * T + j: k * T + j + SC])
                            a_new.append(at[:, :])
                        a_cur = a_new
                    pts = []
                    for k in range(NK):
                        pt = jw.tile([128, SC], bf16, tag="P")
                        eng = nc.gpsimd if k < 4 else nc.vector
                        eng.tensor_add(pt[:, :],
                                       p1b[:, k * SC:(k + 1) * SC],
                                       p2s[:, k * T + j: k * T + j + SC])
                        pts.append(pt)
                    for hf in range(2):
                        pb = jpsum.tile([128, H], f32, tag="pb")
                        for k in range(NK):
                            w = pb[:, k * 128:(k + 1) * 128]
                            nc.tensor.matmul(
                                w, pts[k][:, hf * 128:(hf + 1) * 128],
                                idn[:, 0:128], start=True, stop=False)
                            nc.tensor.matmul(
                                w, a_cur[k][:, hf * 128:(hf + 1) * 128],
                                idn[:, j * 128:(j + 1) * 128],
                                start=False, stop=True)
                        st = stpool.tile([128, H], f32, tag="st")
                        nc.scalar.activation(st[:, :], pb[:, :],
                                             mybir.ActivationFunctionType.Tanh)
                        nc.sync.dma_start(
                            out_d[hf * 128:(hf + 1) * 128,
                                  j * H:(j + 1) * H], st[:, :])
    nc.compile()
    return nc


def _prep_inputs(seq_hiddens, W, b):
    import ml_dtypes
    bf16 = ml_dtypes.bfloat16
    x = np.asarray(seq_hiddens, dtype=np.float32)
    Wn = np.asarray(W, dtype=np.float32)
    bn = np.asarray(b, dtype=np.float32).reshape(H, 1)

    xpad = np.pad(x, ((0, 0), (0, V - 1), (0, 0)))          # [B, S+31, H]
    # wT[p, k*128+h, o] = W[o, p*768 + k*128 + h]
    wT = np.ascontiguousarray(
        Wn.reshape(H, 3, NK * 128).transpose(1, 2, 0)).reshape(3 * NK * 128, H)
    wT = wT.astype(bf16)
    r = (1.0 / np.arange(1, V + 1, dtype=np.float32))
    idn = np.zeros((V, 128, 128), np.float32)
    for j in range(V):
        np.fill_diagonal(idn[j], r[j])
    idn = idn.reshape(V * 128, 128).astype(bf16)

    in_maps = []
    for core in range(8):
        bb, half = core // 2, core % 2
        sl = xpad[bb, half * SC: half * SC + T, :]           # [287, 768]
        xT = np.ascontiguousarray(sl.T).astype(bf16)         # [768, 287]
        in_maps.append({"xT": xT, "wT": wT, "idn": idn, "bv": bn})
    return in_maps


_TAIL_IDX = None


def _tail_index():
    global _TAIL_IDX
    if _TAIL_IDX is None:
        idx = [s * 32 + j for s in range(225, 256) for j in range(256 - s)]
        _TAIL_IDX = np.asarray(idx, dtype=np.int64)
    return _TAIL_IDX


def _assemble(results):
    out = np.empty((B, 15888, H), np.float32)
    for bb in range(B):
        h0 = results[2 * bb]["out"].reshape(SC * V, H)
        h1 = results[2 * bb + 1]["out"].reshape(SC * V, H)
        out[bb, :8192] = h0
        out[bb, 8192:15392] = h1[:7200]
        out[bb, 15392:] = h1[_tail_index()]
    return out


def _install_ntff_hook():
    """Register the axon NTFF-profile hook (missing from the antenv stub)."""
    import sys
    if "antenv.axon_hooks" in sys.modules:
        return
    import contextlib
    import ctypes
    import types

    so_path = "/opt/axon/libaxon_pjrt.so"
    lib = ctypes.CDLL(so_path)
    if not hasattr(lib, "axon_start_nrt_profile"):
        return
    lib.axon_start_nrt_profile.argtypes = [ctypes.POINTER(ctypes.c_int64),
                                           ctypes.c_size_t]
    lib.axon_start_nrt_profile.restype = ctypes.c_int64
    lib.axon_stop_nrt_profile.argtypes = [ctypes.c_char_p]
    lib.axon_stop_nrt_profile.restype = ctypes.c_int64

    @contextlib.contextmanager
    def _hook(output_dir, device_ids):
        import jax
        jax.devices()
        if device_ids:
            ids = (ctypes.c_int64 * len(device_ids))(*device_ids)
            rc = lib.axon_start_nrt_profile(ids, len(device_ids))
        else:
            rc = lib.axon_start_nrt_profile(None, 0)
        if rc != 0:
            raise RuntimeError(f"axon_start_nrt_profile rc={rc}")
        try:
            yield
        finally:
            n = lib.axon_stop_nrt_profile(str(output_dir).encode())
            print(f"profile: {n} file(s) written to {output_dir}", file=sys.stderr)

    mod = types.ModuleType("antenv.axon_hooks")
    mod.get_axon_ntff_profile_hook = lambda: _hook
    mod.set_axon_ntff_profile_hook = lambda h: None
    sys.modules["antenv.axon_hooks"] = mod


def run_hw(seq_hiddens, W, b, trace=False):
    from concourse.bass_utils import run_bass_kernel_spmd
    if trace:
        _install_ntff_hook()
    if "nc" not in _CACHE:
        _CACHE["nc"] = _build_program()
    nc = _CACHE["nc"]
    in_maps = _prep_inputs(seq_hiddens, W, b)
    res = run_bass_kernel_spmd(nc, in_maps, list(range(8)), trace=trace)
    return _assemble(res.results), res


def _compute_np(seq_hiddens, W, b):
    x = np.asarray(seq_hiddens, dtype=np.float32)
    Wn = np.asarray(W, dtype=np.float32)
    bn = np.asarray(b, dtype=np.float32)
    idx = np.arange(S)[:, None] + np.arange(V)[None, :]
    mask = idx < S
    si, ji = np.nonzero(mask)
    padded = np.pad(x, ((0, 0), (0, V - 1), (0, 0)))
    visual = padded[:, idx, :]
    denom = np.arange(1, V + 1, dtype=np.float32)[None, None, :, None]
    context = np.cumsum(visual, axis=2, dtype=np.float32) / denom
    W1, W2, W3 = Wn[:, :H], Wn[:, H:2 * H], Wn[:, 2 * H:]
    rep = x @ W1.T
    vis = (visual.reshape(-1, H) @ W2.T).reshape(B, S, V, H)
    ctx = (context.reshape(-1, H) @ W3.T).reshape(B, S, V, H)
    out = np.tanh(rep[:, :, None, :] + vis + ctx + bn)
    return np.ascontiguousarray(out[:, si, ji, :].astype(np.float32))


def kernel(seq_hiddens, W, b):
    try:
        out, _ = run_hw(seq_hiddens, W, b, trace=False)
        return out
    except Exception:
        return _compute_np(seq_hiddens, W, b)


# revision 8
# speedup vs baseline: 150788.5868x; 1.3671x over previous
"""Handshaking kernel on 8 Trainium2 NeuronCores via Bass/Tile.

Math (per batch b, start s, window offset j < 32, feature o):
  out[b, s, j, o] = tanh( p1[b,s,o] + p2[b,s+j,o]
                          + (1/(j+1)) * sum_{u=s}^{s+j} p3[b,u,o] + bias[o] )
with pk[t] = x[t] @ Wk.T,  Wk = W[:, k*768:(k+1)*768]  (W indexed [o, h]).

Sharding: 8 cores, each takes 256 consecutive starts of one batch element
(core = 2*b + half).  The windowed terms only need a 31-row forward halo,
so every core gets an independent [288, 768] slice of x -- no collectives.

On-core layout is feature-major ([o partitions, t free]) so the window
shifts are free strided views.  Per (s,j) output rows are produced on the
tensor engine as "scaled transpose" accumulation into PSUM:
    psum = p1b.T @ I + p2[.,s+j].T @ I + A_j.T @ (I/(j+1))
(A_j = running window sum of p3, one fused VectorE add per j), which
ScalarE evacuates with fused tanh into natural layout for contiguous
12KB-per-partition DMA writes.
"""

import numpy as np

B, S, H, V = 4, 512, 768, 32
SC = 256          # starts per core
T = 288           # halo'd positions per core (287 real + 1 pad, even stride)
NK = H // 128     # 6 feature tiles
JG = 4            # j's per output DMA group

_CACHE = {}


def _build_program():
    import concourse.bacc as bacc
    import concourse.bass as bass
    import concourse.mybir as mybir
    import concourse.tile as tile

    f32 = mybir.dt.float32
    f16 = mybir.dt.float16

    nc = bacc.Bacc("TRN2", target_bir_lowering=False, debug=False,
                   enable_asserts=False, num_devices=8)

    xT_d = nc.dram_tensor("xT", [NK * 128, T], f16, kind="ExternalInput")
    wT_d = nc.dram_tensor("wT", [3 * NK * 128, H], f16, kind="ExternalInput")
    idn_d = nc.dram_tensor("idn", [V * 128, 128], f16, kind="ExternalInput")
    bv_d = nc.dram_tensor("bv", [NK * 128, 1], f32, kind="ExternalInput")
    out_d = nc.dram_tensor("out", [SC, V * H], f32, kind="ExternalOutput")

    with tile.TileContext(nc) as tc:
        with tc.tile_pool(name="persist", bufs=1) as persist:
            xt = persist.tile([128, NK * T], f16, tag="xt")
            wt = persist.tile([128, 3 * NK * H], f16, tag="wt")
            idn = persist.tile([128, V * 128], f16, tag="idn")
            bv = persist.tile([128, NK], f32, tag="bv")
            p1b = persist.tile([128, NK * SC], f16, tag="p1b")
            p2s = persist.tile([128, NK * T], f16, tag="p2s")
            p3s = persist.tile([128, NK * T], f16, tag="p3s")
            p3o = persist.tile([128, NK * T], f16, tag="p3o")  # p3 shifted by 1

            nc.sync.dma_start(
                xt[:, :].rearrange("p (k t) -> p k t", k=NK),
                xT_d[:, :].rearrange("(k p) t -> p k t", k=NK))
            nc.sync.dma_start(
                wt[:, :].rearrange("p (m o) -> p m o", m=3 * NK),
                wT_d[:, :].rearrange("(m p) o -> p m o", m=3 * NK))
            nc.sync.dma_start(
                idn[:, :].rearrange("p (j q) -> p j q", j=V),
                idn_d[:, :].rearrange("(j p) q -> p j q", j=V))
            nc.sync.dma_start(
                bv[:, :].rearrange("p (k o) -> p k o", k=NK),
                bv_d[:, :].rearrange("(k p) o -> p k o", k=NK))

            # ---- projections: p[o_tile i, t] = sum_k W[.,i].T @ x[k] ----
            with tc.tile_pool(name="ppsum", bufs=4, space="PSUM") as ppsum:
                for p in range(3):
                    for i in range(NK):
                        ps = ppsum.tile([128, T], f32, tag="ps")
                        for k in range(NK):
                            wcol = wt[:, (p * NK + k) * H + i * 128:
                                      (p * NK + k) * H + (i + 1) * 128]
                            nc.tensor.matmul(ps[:, :], wcol,
                                             xt[:, k * T:(k + 1) * T],
                                             start=(k == 0), stop=(k == NK - 1))
                        if p == 0:
                            nc.vector.tensor_scalar_add(
                                p1b[:, i * SC:(i + 1) * SC],
                                ps[:, 0:SC], bv[:, i:i + 1])
                        elif p == 1:
                            nc.scalar.copy(p2s[:, i * T:(i + 1) * T], ps[:, :])
                        else:
                            nc.scalar.copy(p3s[:, i * T:(i + 1) * T], ps[:, :])
                            nc.vector.tensor_copy(p3o[:, i * T:i * T + T - 1],
                                                  ps[:, 1:T])

            # ---- j loop ----
            with tc.tile_pool(name="jw", bufs=3) as jw, \
                 tc.tile_pool(name="jpsum", bufs=4, space="PSUM") as jpsum, \
                 tc.tile_pool(name="stage", bufs=4) as stpool:
                a_cur = p3s
                a_off = 0            # A_j view = a_cur[:, k*stride + a_off ...]
                a_stride = T
                stage_cur = [None, None]
                for j in range(V):
                    if j > 0:
                        at = jw.tile([128, NK * SC], f16, tag="A")
                        src, off = (p3s, j) if j % 2 == 0 else (p3o, j - 1)
                        nc.vector.tensor_tensor(
                            at[:, :].rearrange("p (k t) -> p k t", k=NK),
                            a_cur[:, :].rearrange(
                                "p (k t) -> p k t", k=NK, t=a_stride
                            )[:, :, a_off:a_off + SC],
                            src[:, :].rearrange(
                                "p (k t) -> p k t", k=NK
                            )[:, :, off:off + SC],
                            op=mybir.AluOpType.add)
                        a_cur, a_off, a_stride = at, 0, SC
                    if j % JG == 0:
                        stage_cur = [stpool.tile([128, JG * H], f32, tag="st",
                                                 name=f"st{j}_{hh}")
                                     for hh in range(2)]
                    for hf in range(2):
                        pb = jpsum.tile([128, H], f32, tag="pb")
                        for k in range(NK):
                            w = pb[:, k * 128:(k + 1) * 128]
                            nc.tensor.matmul(
                                w, p1b[:, k * SC + hf * 128:
                                       k * SC + hf * 128 + 128],
                                idn[:, 0:128], start=True, stop=False)
                            nc.tensor.matmul(
                                w, p2s[:, k * T + j + hf * 128:
                                       k * T + j + hf * 128 + 128],
                                idn[:, 0:128], start=False, stop=False)
                            nc.tensor.matmul(
                                w, a_cur[:, k * a_stride + a_off + hf * 128:
                                         k * a_stride + a_off + hf * 128 + 128],
                                idn[:, j * 128:(j + 1) * 128],
                                start=False, stop=True)
                        nc.scalar.activation(
                            stage_cur[hf][:, (j % JG) * H:(j % JG + 1) * H],
                            pb[:, :], mybir.ActivationFunctionType.Tanh)
                    if j % JG == JG - 1:
                        for hf in range(2):
                            nc.sync.dma_start(
                                out_d[hf * 128:(hf + 1) * 128,
                                      (j - JG + 1) * H:(j + 1) * H],
                                stage_cur[hf][:, :])
    nc.compile()
    return nc


def _prep_inputs(seq_hiddens, W, b):
    x = np.asarray(seq_hiddens, dtype=np.float32)
    Wn = np.asarray(W, dtype=np.float32)
    bn = np.asarray(b, dtype=np.float32).reshape(H, 1)

    xpad = np.pad(x, ((0, 0), (0, T - SC), (0, 0)))          # [B, S+32, H]
    # wT[p, k*128+h, o] = W[o, p*768 + k*128 + h]
    wT = np.ascontiguousarray(
        Wn.reshape(H, 3, NK * 128).transpose(1, 2, 0)).reshape(3 * NK * 128, H)
    wT = wT.astype(np.float16)
    r = (1.0 / np.arange(1, V + 1, dtype=np.float32))
    idn = np.zeros((V, 128, 128), np.float32)
    for j in range(V):
        np.fill_diagonal(idn[j], r[j])
    idn = idn.reshape(V * 128, 128).astype(np.float16)

    in_maps = []
    for core in range(8):
        bb, half = core // 2, core % 2
        sl = xpad[bb, half * SC: half * SC + T, :]            # [288, 768]
        xT = np.ascontiguousarray(sl.T).astype(np.float16)    # [768, 288]
        in_maps.append({"xT": xT, "wT": wT, "idn": idn, "bv": bn})
    return in_maps


_TAIL_IDX = None


def _tail_index():
    global _TAIL_IDX
    if _TAIL_IDX is None:
        idx = [s * 32 + j for s in range(225, 256) for j in range(256 - s)]
        _TAIL_IDX = np.asarray(idx, dtype=np.int64)
    return _TAIL_IDX


def _assemble(results):
    out = np.empty((B, 15888, H), np.float32)
    for bb in range(B):
        h0 = results[2 * bb]["out"].reshape(SC * V, H)
        h1 = results[2 * bb + 1]["out"].reshape(SC * V, H)
        out[bb, :8192] = h0
        out[bb, 8192:15392] = h1[:7200]
        out[bb, 15392:] = h1[_tail_index()]
    return out


def _install_ntff_hook():
    """Register the axon NTFF-profile hook (missing from the antenv stub)."""
    import sys
    if "antenv.axon_hooks" in sys.modules:
        return
    import contextlib
    import ctypes
    import types

    so_path = "/opt/axon/libaxon_pjrt.so"
    lib = ctypes.CDLL(so_path)
    if not hasattr(lib, "axon_start_nrt_profile"):
        return
    lib.axon_start_nrt_profile.argtypes = [ctypes.POINTER(ctypes.c_int64),
                                           ctypes.c_size_t]
    lib.axon_start_nrt_profile.restype = ctypes.c_int64
    lib.axon_stop_nrt_profile.argtypes = [ctypes.c_char_p]
    lib.axon_stop_nrt_profile.restype = ctypes.c_int64

    @contextlib.contextmanager
    def _hook(output_dir, device_ids):
        import jax
        jax.devices()
        if device_ids:
            ids = (ctypes.c_int64 * len(device_ids))(*device_ids)
            rc = lib.axon_start_nrt_profile(ids, len(device_ids))
        else:
            rc = lib.axon_start_nrt_profile(None, 0)
        if rc != 0:
            raise RuntimeError(f"axon_start_nrt_profile rc={rc}")
        try:
            yield
        finally:
            n = lib.axon_stop_nrt_profile(str(output_dir).encode())
            print(f"profile: {n} file(s) written to {output_dir}", file=sys.stderr)

    mod = types.ModuleType("antenv.axon_hooks")
    mod.get_axon_ntff_profile_hook = lambda: _hook
    mod.set_axon_ntff_profile_hook = lambda h: None
    sys.modules["antenv.axon_hooks"] = mod


def run_hw(seq_hiddens, W, b, trace=False):
    from concourse.bass_utils import run_bass_kernel_spmd
    if trace:
        _install_ntff_hook()
    if "nc" not in _CACHE:
        _CACHE["nc"] = _build_program()
    nc = _CACHE["nc"]
    in_maps = _prep_inputs(seq_hiddens, W, b)
    res = run_bass_kernel_spmd(nc, in_maps, list(range(8)), trace=trace)
    return _assemble(res.results), res


def _compute_np(seq_hiddens, W, b):
    x = np.asarray(seq_hiddens, dtype=np.float32)
    Wn = np.asarray(W, dtype=np.float32)
    bn = np.asarray(b, dtype=np.float32)
    idx = np.arange(S)[:, None] + np.arange(V)[None, :]
    mask = idx < S
    si, ji = np.nonzero(mask)
    padded = np.pad(x, ((0, 0), (0, V - 1), (0, 0)))
    visual = padded[:, idx, :]
    denom = np.arange(1, V + 1, dtype=np.float32)[None, None, :, None]
    context = np.cumsum(visual, axis=2, dtype=np.float32) / denom
    W1, W2, W3 = Wn[:, :H], Wn[:, H:2 * H], Wn[:, 2 * H:]
    rep = x @ W1.T
    vis = (visual.reshape(-1, H) @ W2.T).reshape(B, S, V, H)
    ctx = (context.reshape(-1, H) @ W3.T).reshape(B, S, V, H)
    out = np.tanh(rep[:, :, None, :] + vis + ctx + bn)
    return np.ascontiguousarray(out[:, si, ji, :].astype(np.float32))


def kernel(seq_hiddens, W, b):
    try:
        out, _ = run_hw(seq_hiddens, W, b, trace=False)
        return out
    except Exception:
        return _compute_np(seq_hiddens, W, b)


# revision 14
# speedup vs baseline: 160079.6652x; 1.0616x over previous
"""Handshaking kernel on 8 Trainium2 NeuronCores via Bass/Tile.

Math (per batch b, start s, window offset j < 32, feature o):
  out[b, s, j, o] = tanh( p1[b,s,o] + p2[b,s+j,o]
                          + (1/(j+1)) * sum_{u=s}^{s+j} p3[b,u,o] + bias[o] )
with pk[t] = x[t] @ Wk.T,  Wk = W[:, k*768:(k+1)*768]  (W indexed [o, h]).

Sharding: 8 cores, each takes 256 consecutive starts of one batch element
(core = 2*b + half).  The windowed terms only need a 31-row forward halo,
so every core gets an independent [288, 768] slice of x -- no collectives.

On-core layout is feature-major ([o partitions, t free]) so the window
shifts are free strided views.  Per (s,j) output rows are produced on the
tensor engine as "scaled transpose" accumulation into PSUM:
    psum = p1b.T @ I + p2[.,s+j].T @ I + A_j.T @ (I/(j+1))
(A_j = running window sum of p3, one fused VectorE add per j), which
ScalarE evacuates with fused tanh into natural layout for contiguous
12KB-per-partition DMA writes.
"""

import numpy as np

B, S, H, V = 4, 512, 768, 32
SC = 256          # starts per core
T = 288           # halo'd positions per core (287 real + 1 pad, even stride)
NK = H // 128     # 6 feature tiles
JG = 2            # j's per output DMA group

_CACHE = {}


def _build_program():
    import concourse.bacc as bacc
    import concourse.bass as bass
    import concourse.mybir as mybir
    import concourse.tile as tile

    f32 = mybir.dt.float32
    f16 = mybir.dt.float16

    nc = bacc.Bacc("TRN2", target_bir_lowering=False, debug=False,
                   enable_asserts=False, num_devices=8)

    xT_d = nc.dram_tensor("xT", [NK * 128, T], f16, kind="ExternalInput")
    wT_d = nc.dram_tensor("wT", [3 * NK * 128, H], f16, kind="ExternalInput")
    idn_d = nc.dram_tensor("idn", [V * 128, 128], f16, kind="ExternalInput")
    bv_d = nc.dram_tensor("bv", [NK * 128, 1], f32, kind="ExternalInput")
    out_d = nc.dram_tensor("out", [SC, V * H], f16, kind="ExternalOutput")

    with tile.TileContext(nc) as tc:
        with tc.tile_pool(name="persist", bufs=1) as persist:
            xt = persist.tile([128, NK * T], f16, tag="xt")
            wt = persist.tile([128, 3 * NK * H], f16, tag="wt")
            idn = persist.tile([128, V * 128], f16, tag="idn")
            bv = persist.tile([128, NK], f32, tag="bv")
            p1b = persist.tile([128, NK * SC], f16, tag="p1b")
            p2s = persist.tile([128, NK * T], f16, tag="p2s")
            p2o = persist.tile([128, NK * T], f16, tag="p2o")  # p2 shifted by 1
            p3s = persist.tile([128, NK * T], f16, tag="p3s")
            p3o = persist.tile([128, NK * T], f16, tag="p3o")  # p3 shifted by 1

            nc.sync.dma_start(
                xt[:, :].rearrange("p (k t) -> p k t", k=NK),
                xT_d[:, :].rearrange("(k p) t -> p k t", k=NK))
            for p in range(3):
                nc.sync.dma_start(
                    wt[:, p * NK * H:(p + 1) * NK * H].rearrange(
                        "p (m o) -> p m o", m=NK),
                    wT_d[p * NK * 128:(p + 1) * NK * 128, :].rearrange(
                        "(m p) o -> p m o", m=NK))
            nc.sync.dma_start(
                idn[:, :].rearrange("p (j q) -> p j q", j=V),
                idn_d[:, :].rearrange("(j p) q -> p j q", j=V))
            nc.sync.dma_start(
                bv[:, :].rearrange("p (k o) -> p k o", k=NK),
                bv_d[:, :].rearrange("(k p) o -> p k o", k=NK))

            # ---- projections: p[o_tile i, t] = sum_k W[.,i].T @ x[k] ----
            with tc.tile_pool(name="ppsum", bufs=4, space="PSUM") as ppsum:
                for p in range(3):
                    for i in range(NK):
                        ps = ppsum.tile([128, T], f32, tag="ps")
                        for k in range(NK):
                            wcol = wt[:, (p * NK + k) * H + i * 128:
                                      (p * NK + k) * H + (i + 1) * 128]
                            nc.tensor.matmul(ps[:, :], wcol,
                                             xt[:, k * T:(k + 1) * T],
                                             start=(k == 0), stop=(k == NK - 1))
                        if p == 0:
                            nc.vector.tensor_scalar_add(
                                p1b[:, i * SC:(i + 1) * SC],
                                ps[:, 0:SC], bv[:, i:i + 1])
                        elif p == 1:
                            nc.scalar.copy(p2s[:, i * T:(i + 1) * T], ps[:, :])
                            nc.vector.tensor_copy(p2o[:, i * T:i * T + T - 1],
                                                  ps[:, 1:T])
                        else:
                            nc.scalar.copy(p3s[:, i * T:(i + 1) * T], ps[:, :])
                            nc.vector.tensor_copy(p3o[:, i * T:i * T + T - 1],
                                                  ps[:, 1:T])

            # ---- j loop ----
            with tc.tile_pool(name="jw", bufs=3) as jw, \
                 tc.tile_pool(name="jpsum", bufs=4, space="PSUM") as jpsum, \
                 tc.tile_pool(name="stage", bufs=4) as stpool:
                def r3(ap, t=None):
                    return ap[:, :].rearrange("p (k t) -> p k t", k=NK)

                a_cur, a_off, a_stride = p3s, 0, T
                stage_cur = [None, None]
                for j in range(V):
                    if j > 0:
                        at = jw.tile([128, NK * SC], f16, tag="A")
                        src, off = (p3s, j) if j % 2 == 0 else (p3o, j - 1)
                        nc.vector.tensor_tensor(
                            r3(at),
                            r3(a_cur)[:, :, a_off:a_off + SC],
                            r3(src)[:, :, off:off + SC],
                            op=mybir.AluOpType.add)
                        a_cur, a_off, a_stride = at, 0, SC
                    # P = p1b + p2[., s+j]  (one fused DVE add, 2x mode)
                    pt = jw.tile([128, NK * SC], f16, tag="P")
                    p2src, p2off = (p2s, j) if j % 2 == 0 else (p2o, j - 1)
                    nc.vector.tensor_tensor(
                        r3(pt), r3(p1b),
                        r3(p2src)[:, :, p2off:p2off + SC],
                        op=mybir.AluOpType.add)
                    if j % JG == 0:
                        stage_cur = [stpool.tile([128, JG * H], f16, tag="st",
                                                 name=f"st{j}_{hh}")
                                     for hh in range(2)]
                    for hf in range(2):
                        pb = jpsum.tile([128, H], f32, tag="pb")
                        for k in range(NK):
                            w = pb[:, k * 128:(k + 1) * 128]
                            nc.tensor.matmul(
                                w, pt[:, k * SC + hf * 128:
                                      k * SC + hf * 128 + 128],
                                idn[:, 0:128], start=True, stop=False)
                            nc.tensor.matmul(
                                w, a_cur[:, k * a_stride + a_off + hf * 128:
                                         k * a_stride + a_off + hf * 128 + 128],
                                idn[:, j * 128:(j + 1) * 128],
                                start=False, stop=True)
                        nc.scalar.activation(
                            stage_cur[hf][:, (j % JG) * H:(j % JG + 1) * H],
                            pb[:, :], mybir.ActivationFunctionType.Tanh)
                    if j % JG == JG - 1:
                        for hf in range(2):
                            nc.sync.dma_start(
                                out_d[hf * 128:(hf + 1) * 128,
                                      (j - JG + 1) * H:(j + 1) * H],
                                stage_cur[hf][:, :])
    nc.compile()
    return nc


def _prep_inputs(seq_hiddens, W, b):
    x = np.asarray(seq_hiddens, dtype=np.float32)
    Wn = np.asarray(W, dtype=np.float32)
    bn = np.asarray(b, dtype=np.float32).reshape(H, 1)

    xpad = np.pad(x, ((0, 0), (0, T - SC), (0, 0)))          # [B, S+32, H]
    # wT[p, k*128+h, o] = W[o, p*768 + k*128 + h]
    wT = np.ascontiguousarray(
        Wn.reshape(H, 3, NK * 128).transpose(1, 2, 0)).reshape(3 * NK * 128, H)
    wT = wT.astype(np.float16)
    r = (1.0 / np.arange(1, V + 1, dtype=np.float32))
    idn = np.zeros((V, 128, 128), np.float32)
    for j in range(V):
        np.fill_diagonal(idn[j], r[j])
    idn = idn.reshape(V * 128, 128).astype(np.float16)

    in_maps = []
    for core in range(8):
        bb, half = core // 2, core % 2
        sl = xpad[bb, half * SC: half * SC + T, :]            # [288, 768]
        xT = np.ascontiguousarray(sl.T).astype(np.float16)    # [768, 288]
        in_maps.append({"xT": xT, "wT": wT, "idn": idn, "bv": bn})
    return in_maps


_TAIL_IDX = None


def _tail_index():
    global _TAIL_IDX
    if _TAIL_IDX is None:
        idx = [s * 32 + j for s in range(225, 256) for j in range(256 - s)]
        _TAIL_IDX = np.asarray(idx, dtype=np.int64)
    return _TAIL_IDX


def _assemble(results):
    out = np.empty((B, 15888, H), np.float32)
    for bb in range(B):
        h0 = results[2 * bb]["out"].reshape(SC * V, H)
        h1 = results[2 * bb + 1]["out"].reshape(SC * V, H)
        out[bb, :8192] = h0.astype(np.float32)
        out[bb, 8192:15392] = h1[:7200].astype(np.float32)
        out[bb, 15392:] = h1[_tail_index()].astype(np.float32)
    return out


def _install_ntff_hook():
    """Register the axon NTFF-profile hook (missing from the antenv stub)."""
    import sys
    if "antenv.axon_hooks" in sys.modules:
        return
    import contextlib
    import ctypes
    import types

    so_path = "/opt/axon/libaxon_pjrt.so"
    lib = ctypes.CDLL(so_path)
    if not hasattr(lib, "axon_start_nrt_profile"):
        return
    lib.axon_start_nrt_profile.argtypes = [ctypes.POINTER(ctypes.c_int64),
                                           ctypes.c_size_t]
    lib.axon_start_nrt_profile.restype = ctypes.c_int64
    lib.axon_stop_nrt_profile.argtypes = [ctypes.c_char_p]
    lib.axon_stop_nrt_profile.restype = ctypes.c_int64

    @contextlib.contextmanager
    def _hook(output_dir, device_ids):
        import jax
        jax.devices()
        if device_ids:
            ids = (ctypes.c_int64 * len(device_ids))(*device_ids)
            rc = lib.axon_start_nrt_profile(ids, len(device_ids))
        else:
            rc = lib.axon_start_nrt_profile(None, 0)
        if rc != 0:
            raise RuntimeError(f"axon_start_nrt_profile rc={rc}")
        try:
            yield
        finally:
            n = lib.axon_stop_nrt_profile(str(output_dir).encode())
            print(f"profile: {n} file(s) written to {output_dir}", file=sys.stderr)

    mod = types.ModuleType("antenv.axon_hooks")
    mod.get_axon_ntff_profile_hook = lambda: _hook
    mod.set_axon_ntff_profile_hook = lambda h: None
    sys.modules["antenv.axon_hooks"] = mod


def run_hw(seq_hiddens, W, b, trace=False):
    from concourse.bass_utils import run_bass_kernel_spmd
    if trace:
        _install_ntff_hook()
    if "nc" not in _CACHE:
        _CACHE["nc"] = _build_program()
    nc = _CACHE["nc"]
    in_maps = _prep_inputs(seq_hiddens, W, b)
    res = run_bass_kernel_spmd(nc, in_maps, list(range(8)), trace=trace)
    return _assemble(res.results), res


def _compute_np(seq_hiddens, W, b):
    x = np.asarray(seq_hiddens, dtype=np.float32)
    Wn = np.asarray(W, dtype=np.float32)
    bn = np.asarray(b, dtype=np.float32)
    idx = np.arange(S)[:, None] + np.arange(V)[None, :]
    mask = idx < S
    si, ji = np.nonzero(mask)
    padded = np.pad(x, ((0, 0), (0, V - 1), (0, 0)))
    visual = padded[:, idx, :]
    denom = np.arange(1, V + 1, dtype=np.float32)[None, None, :, None]
    context = np.cumsum(visual, axis=2, dtype=np.float32) / denom
    W1, W2, W3 = Wn[:, :H], Wn[:, H:2 * H], Wn[:, 2 * H:]
    rep = x @ W1.T
    vis = (visual.reshape(-1, H) @ W2.T).reshape(B, S, V, H)
    ctx = (context.reshape(-1, H) @ W3.T).reshape(B, S, V, H)
    out = np.tanh(rep[:, :, None, :] + vis + ctx + bn)
    return np.ascontiguousarray(out[:, si, ji, :].astype(np.float32))


def kernel(seq_hiddens, W, b):
    try:
        out, _ = run_hw(seq_hiddens, W, b, trace=False)
        return out
    except Exception:
        return _compute_np(seq_hiddens, W, b)


# revision 18
# speedup vs baseline: 181371.4273x; 1.1330x over previous
"""Handshaking kernel on 8 Trainium2 NeuronCores via Bass/Tile.

Math (per batch b, start s, window offset j < 32, feature o):
  out[b, s, j, o] = tanh( p1[b,s,o] + p2[b,s+j,o]
                          + (1/(j+1)) * sum_{u=s}^{s+j} p3[b,u,o] + bias[o] )
with pk[t] = x[t] @ Wk.T,  Wk = W[:, k*768:(k+1)*768]  (W indexed [o, h]).

Sharding: 8 cores, each takes 256 consecutive starts of one batch element
(core = 2*b + half).  The windowed terms only need a 31-row forward halo,
so every core gets an independent [288, 768] slice of x -- no collectives.

On-core layout is feature-major ([o partitions, t free]) so the window
shifts are free strided views.  Per (s,j) output rows are produced on the
tensor engine as "scaled transpose" accumulation into PSUM:
    psum = p1b.T @ I + p2[.,s+j].T @ I + A_j.T @ (I/(j+1))
(A_j = running window sum of p3, one fused VectorE add per j), which
ScalarE evacuates with fused tanh into natural layout for contiguous
12KB-per-partition DMA writes.
"""

import numpy as np

B, S, H, V = 4, 512, 768, 32
SC = 256          # starts per core
T = 288           # halo'd positions per core (287 real + 1 pad, even stride)
NK = H // 128     # 6 feature tiles
JG = 2            # j's per output DMA group

_CACHE = {}


def _build_program():
    import concourse.bacc as bacc
    import concourse.bass as bass
    import concourse.mybir as mybir
    import concourse.tile as tile

    f32 = mybir.dt.float32
    f16 = mybir.dt.float16

    nc = bacc.Bacc("TRN2", target_bir_lowering=False, debug=False,
                   enable_asserts=False, num_devices=8)

    xT_d = nc.dram_tensor("xT", [NK * 128, T], f16, kind="ExternalInput")
    wT_d = nc.dram_tensor("wT", [3 * NK * 128, H], f16, kind="ExternalInput")
    idn_d = nc.dram_tensor("idn", [V * 128, 128], f16, kind="ExternalInput")
    bv_d = nc.dram_tensor("bv", [NK * 128, 1], f32, kind="ExternalInput")
    out_d = nc.dram_tensor("out", [SC, V * H], f16, kind="ExternalOutput")

    with tile.TileContext(nc) as tc:
        with tc.tile_pool(name="persist", bufs=1) as persist:
            xt = persist.tile([128, NK * T], f16, tag="xt")
            wt = persist.tile([128, 3 * NK * H], f16, tag="wt")
            idn = persist.tile([128, V * 128], f16, tag="idn")
            bv = persist.tile([128, NK], f32, tag="bv")
            p1b = persist.tile([128, NK * SC], f16, tag="p1b")
            p2s = persist.tile([128, NK * T], f16, tag="p2s")
            p2o = persist.tile([128, NK * T], f16, tag="p2o")  # p2 shifted by 1
            p3s = persist.tile([128, NK * T], f16, tag="p3s")
            p3o = persist.tile([128, NK * T], f16, tag="p3o")  # p3 shifted by 1

            nc.sync.dma_start(
                xt[:, :].rearrange("p (k t) -> p k t", k=NK),
                xT_d[:, :].rearrange("(k p) t -> p k t", k=NK))
            for p in range(3):
                for k in range(NK):
                    m = p * NK + k
                    nc.sync.dma_start(wt[:, m * H:(m + 1) * H],
                                      wT_d[m * 128:(m + 1) * 128, :])
            nc.sync.dma_start(
                idn[:, :].rearrange("p (j q) -> p j q", j=V),
                idn_d[:, :].rearrange("(j p) q -> p j q", j=V))
            nc.sync.dma_start(
                bv[:, :].rearrange("p (k o) -> p k o", k=NK),
                bv_d[:, :].rearrange("(k p) o -> p k o", k=NK))

            # ---- projections: p[o_tile i, t] = sum_k W[.,i].T @ x[k] ----
            with tc.tile_pool(name="ppsum", bufs=4, space="PSUM") as ppsum:
                for p in range(3):
                    for i in range(NK):
                        ps = ppsum.tile([128, T], f32, tag="ps")
                        for k in range(NK):
                            wcol = wt[:, (p * NK + k) * H + i * 128:
                                      (p * NK + k) * H + (i + 1) * 128]
                            nc.tensor.matmul(ps[:, :], wcol,
                                             xt[:, k * T:(k + 1) * T],
                                             start=(k == 0), stop=(k == NK - 1))
                        if p == 0:
                            nc.vector.tensor_scalar_add(
                                p1b[:, i * SC:(i + 1) * SC],
                                ps[:, 0:SC], bv[:, i:i + 1])
                        elif p == 1:
                            nc.scalar.copy(p2s[:, i * T:(i + 1) * T], ps[:, :])
                            nc.scalar.copy(p2o[:, i * T:i * T + T - 1],
                                           ps[:, 1:T])
                        else:
                            nc.scalar.copy(p3s[:, i * T:(i + 1) * T], ps[:, :])
                            nc.scalar.copy(p3o[:, i * T:i * T + T - 1],
                                           ps[:, 1:T])

            # ---- j loop ----
            with tc.tile_pool(name="jw", bufs=3) as jw, \
                 tc.tile_pool(name="jpsum", bufs=2, space="PSUM") as jpsum, \
                 tc.tile_pool(name="stage", bufs=4) as stpool:
                def r3(ap, t=None):
                    return ap[:, :].rearrange("p (k t) -> p k t", k=NK)

                a_cur, a_off, a_stride = p3s, 0, T
                for j0 in range(0, V, 2):
                    avs, pvs = [], []   # (tile, off, stride) per j of the pair
                    for j in (j0, j0 + 1):
                        if j > 0:
                            at = jw.tile([128, NK * SC], f16, tag="A",
                                         name=f"A{j}")
                            src, off = (p3s, j) if j % 2 == 0 else (p3o, j - 1)
                            nc.vector.tensor_tensor(
                                r3(at),
                                r3(a_cur)[:, :, a_off:a_off + SC],
                                r3(src)[:, :, off:off + SC],
                                op=mybir.AluOpType.add)
                            a_cur, a_off, a_stride = at, 0, SC
                        avs.append((a_cur, a_off, a_stride))
                        # P = p1b + p2[., s+j]  (one fused DVE add, 2x mode)
                        pt = jw.tile([128, NK * SC], f16, tag="P",
                                     name=f"P{j}")
                        p2src, p2off = (p2s, j) if j % 2 == 0 else (p2o, j - 1)
                        nc.vector.tensor_tensor(
                            r3(pt), r3(p1b),
                            r3(p2src)[:, :, p2off:p2off + SC],
                            op=mybir.AluOpType.add)
                        pvs.append((pt, 0, SC))
                    for hf in range(2):
                        pb = jpsum.tile([128, 2 * H], f32, tag="pb")
                        for jj in range(2):
                            j = j0 + jj
                            (ac, ao, ast), (pc, po, pst) = avs[jj], pvs[jj]
                            for k in range(NK):
                                w = pb[:, jj * H + k * 128:
                                       jj * H + (k + 1) * 128]
                                nc.tensor.matmul(
                                    w, pc[:, k * pst + po + hf * 128:
                                          k * pst + po + hf * 128 + 128],
                                    idn[:, 0:128], start=True, stop=False)
                                nc.tensor.matmul(
                                    w, ac[:, k * ast + ao + hf * 128:
                                          k * ast + ao + hf * 128 + 128],
                                    idn[:, j * 128:(j + 1) * 128],
                                    start=False, stop=True)
                        st = stpool.tile([128, 2 * H], f16, tag="st",
                                         name=f"st{j0}_{hf}")
                        nc.scalar.activation(
                            st[:, :], pb[:, :],
                            mybir.ActivationFunctionType.Tanh)
                        nc.sync.dma_start(
                            out_d[hf * 128:(hf + 1) * 128,
                                  j0 * H:(j0 + 2) * H], st[:, :])
    nc.compile()
    return nc


def _prep_inputs(seq_hiddens, W, b):
    x = np.asarray(seq_hiddens, dtype=np.float32)
    Wn = np.asarray(W, dtype=np.float32)
    bn = np.asarray(b, dtype=np.float32).reshape(H, 1)

    xpad = np.pad(x, ((0, 0), (0, T - SC), (0, 0)))          # [B, S+32, H]
    # wT[p, k*128+h, o] = W[o, p*768 + k*128 + h]
    wT = np.ascontiguousarray(
        Wn.reshape(H, 3, NK * 128).transpose(1, 2, 0)).reshape(3 * NK * 128, H)
    wT = wT.astype(np.float16)
    r = (1.0 / np.arange(1, V + 1, dtype=np.float32))
    idn = np.zeros((V, 128, 128), np.float32)
    for j in range(V):
        np.fill_diagonal(idn[j], r[j])
    idn = idn.reshape(V * 128, 128).astype(np.float16)

    in_maps = []
    for core in range(8):
        bb, half = core // 2, core % 2
        sl = xpad[bb, half * SC: half * SC + T, :]            # [288, 768]
        xT = np.ascontiguousarray(sl.T).astype(np.float16)    # [768, 288]
        in_maps.append({"xT": xT, "wT": wT, "idn": idn, "bv": bn})
    return in_maps


_TAIL_IDX = None


def _tail_index():
    global _TAIL_IDX
    if _TAIL_IDX is None:
        idx = [s * 32 + j for s in range(225, 256) for j in range(256 - s)]
        _TAIL_IDX = np.asarray(idx, dtype=np.int64)
    return _TAIL_IDX


def _assemble(results):
    out = np.empty((B, 15888, H), np.float32)
    for bb in range(B):
        h0 = results[2 * bb]["out"].reshape(SC * V, H)
        h1 = results[2 * bb + 1]["out"].reshape(SC * V, H)
        out[bb, :8192] = h0.astype(np.float32)
        out[bb, 8192:15392] = h1[:7200].astype(np.float32)
        out[bb, 15392:] = h1[_tail_index()].astype(np.float32)
    return out


def _install_ntff_hook():
    """Register the axon NTFF-profile hook (missing from the antenv stub)."""
    import sys
    if "antenv.axon_hooks" in sys.modules:
        return
    import contextlib
    import ctypes
    import types

    so_path = "/opt/axon/libaxon_pjrt.so"
    lib = ctypes.CDLL(so_path)
    if not hasattr(lib, "axon_start_nrt_profile"):
        return
    lib.axon_start_nrt_profile.argtypes = [ctypes.POINTER(ctypes.c_int64),
                                           ctypes.c_size_t]
    lib.axon_start_nrt_profile.restype = ctypes.c_int64
    lib.axon_stop_nrt_profile.argtypes = [ctypes.c_char_p]
    lib.axon_stop_nrt_profile.restype = ctypes.c_int64

    @contextlib.contextmanager
    def _hook(output_dir, device_ids):
        import jax
        jax.devices()
        if device_ids:
            ids = (ctypes.c_int64 * len(device_ids))(*device_ids)
            rc = lib.axon_start_nrt_profile(ids, len(device_ids))
        else:
            rc = lib.axon_start_nrt_profile(None, 0)
        if rc != 0:
            raise RuntimeError(f"axon_start_nrt_profile rc={rc}")
        try:
            yield
        finally:
            n = lib.axon_stop_nrt_profile(str(output_dir).encode())
            print(f"profile: {n} file(s) written to {output_dir}", file=sys.stderr)

    mod = types.ModuleType("antenv.axon_hooks")
    mod.get_axon_ntff_profile_hook = lambda: _hook
    mod.set_axon_ntff_profile_hook = lambda h: None
    sys.modules["antenv.axon_hooks"] = mod


def run_hw(seq_hiddens, W, b, trace=False):
    from concourse.bass_utils import run_bass_kernel_spmd
    if trace:
        _install_ntff_hook()
    if "nc" not in _CACHE:
        _CACHE["nc"] = _build_program()
    nc = _CACHE["nc"]
    in_maps = _prep_inputs(seq_hiddens, W, b)
    res = run_bass_kernel_spmd(nc, in_maps, list(range(8)), trace=trace)
    return _assemble(res.results), res


def _compute_np(seq_hiddens, W, b):
    x = np.asarray(seq_hiddens, dtype=np.float32)
    Wn = np.asarray(W, dtype=np.float32)
    bn = np.asarray(b, dtype=np.float32)
    idx = np.arange(S)[:, None] + np.arange(V)[None, :]
    mask = idx < S
    si, ji = np.nonzero(mask)
    padded = np.pad(x, ((0, 0), (0, V - 1), (0, 0)))
    visual = padded[:, idx, :]
    denom = np.arange(1, V + 1, dtype=np.float32)[None, None, :, None]
    context = np.cumsum(visual, axis=2, dtype=np.float32) / denom
    W1, W2, W3 = Wn[:, :H], Wn[:, H:2 * H], Wn[:, 2 * H:]
    rep = x @ W1.T
    vis = (visual.reshape(-1, H) @ W2.T).reshape(B, S, V, H)
    ctx = (context.reshape(-1, H) @ W3.T).reshape(B, S, V, H)
    out = np.tanh(rep[:, :, None, :] + vis + ctx + bn)
    return np.ascontiguousarray(out[:, si, ji, :].astype(np.float32))


def kernel(seq_hiddens, W, b):
    try:
        out, _ = run_hw(seq_hiddens, W, b, trace=False)
        return out
    except Exception:
        return _compute_np(seq_hiddens, W, b)


# revision 22
# speedup vs baseline: 200587.6810x; 1.1059x over previous
"""Handshaking kernel on 8 Trainium2 NeuronCores via Bass/Tile.

Math (per batch b, start s, window offset j < 32, feature o):
  out[b, s, j, o] = tanh( p1[b,s,o] + p2[b,s+j,o]
                          + (1/(j+1)) * sum_{u=s}^{s+j} p3[b,u,o] + bias[o] )
with pk[t] = x[t] @ Wk.T,  Wk = W[:, k*768:(k+1)*768]  (W indexed [o, h]).

Sharding: 8 cores, each takes 256 consecutive starts of one batch element
(core = 2*b + half).  The windowed terms only need a 31-row forward halo,
so every core gets an independent [288, 768] slice of x -- no collectives.

On-core layout is feature-major ([o partitions, t free]) so the window
shifts are free strided views.  Per (s,j) output rows are produced on the
tensor engine as "scaled transpose" accumulation into PSUM:
    psum = p1b.T @ I + p2[.,s+j].T @ I + A_j.T @ (I/(j+1))
(A_j = running window sum of p3, one fused VectorE add per j), which
ScalarE evacuates with fused tanh into natural layout for contiguous
12KB-per-partition DMA writes.
"""

import numpy as np

B, S, H, V = 4, 512, 768, 32
SC = 256          # starts per core
T = 288           # halo'd positions per core (287 real + 1 pad, even stride)
NK = H // 128     # 6 feature tiles
JG = 2            # j's per output DMA group

_CACHE = {}


def _build_program():
    import concourse.bacc as bacc
    import concourse.bass as bass
    import concourse.mybir as mybir
    import concourse.tile as tile

    f32 = mybir.dt.float32
    f16 = mybir.dt.float16

    nc = bacc.Bacc("TRN2", target_bir_lowering=False, debug=False,
                   enable_asserts=False, num_devices=8)

    xT_d = nc.dram_tensor("xT", [NK * 128, T], f16, kind="ExternalInput")
    wT_d = nc.dram_tensor("wT", [3 * NK * 128, H], f16, kind="ExternalInput")
    idn_d = nc.dram_tensor("idn", [128, 128], f16, kind="ExternalInput")
    bv_d = nc.dram_tensor("bv", [NK * 128, 1], f32, kind="ExternalInput")
    out_d = nc.dram_tensor("out", [SC, V * H], f16, kind="ExternalOutput")

    with tile.TileContext(nc) as tc:
        with tc.tile_pool(name="persist", bufs=1) as persist:
            xt = persist.tile([128, NK * T], f16, tag="xt")
            wt = persist.tile([128, 3 * NK * H], f16, tag="wt")
            idn = persist.tile([128, V * 128], f16, tag="idn")
            bv = persist.tile([128, NK], f32, tag="bv")
            p1b = persist.tile([128, NK * SC], f16, tag="p1b")
            p2s = persist.tile([128, NK * T], f16, tag="p2s")
            p2o = persist.tile([128, NK * T], f16, tag="p2o")  # p2 shifted by 1
            p3s = persist.tile([128, NK * T], f16, tag="p3s")
            p3o = persist.tile([128, NK * T], f16, tag="p3o")  # p3 shifted by 1

            nc.sync.dma_start(
                bv[:, :].rearrange("p (k o) -> p k o", k=NK),
                bv_d[:, :].rearrange("(k p) o -> p k o", k=NK))
            nc.sync.dma_start(
                xt[:, :].rearrange("p (k t) -> p k t", k=NK),
                xT_d[:, :].rearrange("(k p) t -> p k t", k=NK))
            nc.sync.dma_start(idn[:, 0:128], idn_d[:, :])
            for j in range(1, V):
                nc.vector.tensor_scalar_mul(idn[:, j * 128:(j + 1) * 128],
                                            idn[:, 0:128], 1.0 / (j + 1))
            for p in range(3):
                for k in range(NK):
                    m = p * NK + k
                    nc.sync.dma_start(wt[:, m * H:(m + 1) * H],
                                      wT_d[m * 128:(m + 1) * 128, :])

            # ---- projections: p[o_tile i, t] = sum_k W[.,i].T @ x[k] ----
            with tc.tile_pool(name="ppsum", bufs=4, space="PSUM") as ppsum:
                for p in range(3):
                    for i in range(NK):
                        ps = ppsum.tile([128, T], f32, tag="ps")
                        for k in range(NK):
                            wcol = wt[:, (p * NK + k) * H + i * 128:
                                      (p * NK + k) * H + (i + 1) * 128]
                            nc.tensor.matmul(ps[:, :], wcol,
                                             xt[:, k * T:(k + 1) * T],
                                             start=(k == 0), stop=(k == NK - 1))
                        if p == 0:
                            nc.vector.tensor_scalar_add(
                                p1b[:, i * SC:(i + 1) * SC],
                                ps[:, 0:SC], bv[:, i:i + 1])
                        elif p == 1:
                            nc.vector.tensor_copy(p2s[:, i * T:(i + 1) * T],
                                                  ps[:, :])
                            nc.vector.tensor_copy(p2o[:, i * T:i * T + T - 1],
                                                  ps[:, 1:T])
                        else:
                            nc.vector.tensor_copy(p3s[:, i * T:(i + 1) * T],
                                                  ps[:, :])
                            nc.vector.tensor_copy(p3o[:, i * T:i * T + T - 1],
                                                  ps[:, 1:T])

            # ---- j loop ----
            with tc.tile_pool(name="jw", bufs=3) as jw, \
                 tc.tile_pool(name="jpsum", bufs=2, space="PSUM") as jpsum, \
                 tc.tile_pool(name="stage", bufs=4) as stpool:
                def r3(ap, t=None):
                    return ap[:, :].rearrange("p (k t) -> p k t", k=NK)

                a_cur, a_off, a_stride = p3s, 0, T
                for j0 in range(0, V, 2):
                    avs, pvs = [], []   # (tile, off, stride) per j of the pair
                    for j in (j0, j0 + 1):
                        if j > 0:
                            at = jw.tile([128, NK * SC], f16, tag="A",
                                         name=f"A{j}")
                            src, off = (p3s, j) if j % 2 == 0 else (p3o, j - 1)
                            nc.vector.tensor_tensor(
                                r3(at),
                                r3(a_cur)[:, :, a_off:a_off + SC],
                                r3(src)[:, :, off:off + SC],
                                op=mybir.AluOpType.add)
                            a_cur, a_off, a_stride = at, 0, SC
                        avs.append((a_cur, a_off, a_stride))
                        # P = p1b + p2[., s+j]  (one fused DVE add, 2x mode)
                        pt = jw.tile([128, NK * SC], f16, tag="P",
                                     name=f"P{j}")
                        p2src, p2off = (p2s, j) if j % 2 == 0 else (p2o, j - 1)
                        nc.vector.tensor_tensor(
                            r3(pt), r3(p1b),
                            r3(p2src)[:, :, p2off:p2off + SC],
                            op=mybir.AluOpType.add)
                        pvs.append((pt, 0, SC))
                    for hf in range(2):
                        pb = jpsum.tile([128, 2 * H], f32, tag="pb")
                        for jj in range(2):
                            j = j0 + jj
                            (ac, ao, ast), (pc, po, pst) = avs[jj], pvs[jj]
                            for k in range(NK):
                                w = pb[:, jj * H + k * 128:
                                       jj * H + (k + 1) * 128]
                                nc.tensor.matmul(
                                    w, pc[:, k * pst + po + hf * 128:
                                          k * pst + po + hf * 128 + 128],
                                    idn[:, 0:128], start=True, stop=False)
                                nc.tensor.matmul(
                                    w, ac[:, k * ast + ao + hf * 128:
                                          k * ast + ao + hf * 128 + 128],
                                    idn[:, j * 128:(j + 1) * 128],
                                    start=False, stop=True)
                        st = stpool.tile([128, 2 * H], f16, tag="st",
                                         name=f"st{j0}_{hf}")
                        nc.scalar.activation(
                            st[:, :], pb[:, :],
                            mybir.ActivationFunctionType.Tanh)
                        nc.sync.dma_start(
                            out_d[hf * 128:(hf + 1) * 128,
                                  j0 * H:(j0 + 2) * H], st[:, :])
    nc.compile()
    return nc


def _prep_inputs(seq_hiddens, W, b):
    x = np.asarray(seq_hiddens, dtype=np.float32)
    Wn = np.asarray(W, dtype=np.float32)
    bn = np.asarray(b, dtype=np.float32).reshape(H, 1)

    xpad = np.pad(x, ((0, 0), (0, T - SC), (0, 0)))          # [B, S+32, H]
    # wT[p, k*128+h, o] = W[o, p*768 + k*128 + h]
    wT = np.ascontiguousarray(
        Wn.reshape(H, 3, NK * 128).transpose(1, 2, 0)).reshape(3 * NK * 128, H)
    wT = wT.astype(np.float16)
    idn = np.eye(128, dtype=np.float16)

    in_maps = []
    for core in range(8):
        bb, half = core // 2, core % 2
        sl = xpad[bb, half * SC: half * SC + T, :]            # [288, 768]
        xT = np.ascontiguousarray(sl.T).astype(np.float16)    # [768, 288]
        in_maps.append({"xT": xT, "wT": wT, "idn": idn, "bv": bn})
    return in_maps


_TAIL_IDX = None


def _tail_index():
    global _TAIL_IDX
    if _TAIL_IDX is None:
        idx = [s * 32 + j for s in range(225, 256) for j in range(256 - s)]
        _TAIL_IDX = np.asarray(idx, dtype=np.int64)
    return _TAIL_IDX


def _assemble(results):
    out = np.empty((B, 15888, H), np.float32)
    for bb in range(B):
        h0 = results[2 * bb]["out"].reshape(SC * V, H)
        h1 = results[2 * bb + 1]["out"].reshape(SC * V, H)
        out[bb, :8192] = h0.astype(np.float32)
        out[bb, 8192:15392] = h1[:7200].astype(np.float32)
        out[bb, 15392:] = h1[_tail_index()].astype(np.float32)
    return out


def _install_ntff_hook():
    """Register the axon NTFF-profile hook (missing from the antenv stub)."""
    import sys
    if "antenv.axon_hooks" in sys.modules:
        return
    import contextlib
    import ctypes
    import types

    so_path = "/opt/axon/libaxon_pjrt.so"
    lib = ctypes.CDLL(so_path)
    if not hasattr(lib, "axon_start_nrt_profile"):
        return
    lib.axon_start_nrt_profile.argtypes = [ctypes.POINTER(ctypes.c_int64),
                                           ctypes.c_size_t]
    lib.axon_start_nrt_profile.restype = ctypes.c_int64
    lib.axon_stop_nrt_profile.argtypes = [ctypes.c_char_p]
    lib.axon_stop_nrt_profile.restype = ctypes.c_int64

    @contextlib.contextmanager
    def _hook(output_dir, device_ids):
        import jax
        jax.devices()
        if device_ids:
            ids = (ctypes.c_int64 * len(device_ids))(*device_ids)
            rc = lib.axon_start_nrt_profile(ids, len(device_ids))
        else:
            rc = lib.axon_start_nrt_profile(None, 0)
        if rc != 0:
            raise RuntimeError(f"axon_start_nrt_profile rc={rc}")
        try:
            yield
        finally:
            n = lib.axon_stop_nrt_profile(str(output_dir).encode())
            print(f"profile: {n} file(s) written to {output_dir}", file=sys.stderr)

    mod = types.ModuleType("antenv.axon_hooks")
    mod.get_axon_ntff_profile_hook = lambda: _hook
    mod.set_axon_ntff_profile_hook = lambda h: None
    sys.modules["antenv.axon_hooks"] = mod


def run_hw(seq_hiddens, W, b, trace=False):
    from concourse.bass_utils import run_bass_kernel_spmd
    if trace:
        _install_ntff_hook()
    if "nc" not in _CACHE:
        _CACHE["nc"] = _build_program()
    nc = _CACHE["nc"]
    in_maps = _prep_inputs(seq_hiddens, W, b)
    res = run_bass_kernel_spmd(nc, in_maps, list(range(8)), trace=trace)
    return _assemble(res.results), res


def _compute_np(seq_hiddens, W, b):
    x = np.asarray(seq_hiddens, dtype=np.float32)
    Wn = np.asarray(W, dtype=np.float32)
    bn = np.asarray(b, dtype=np.float32)
    idx = np.arange(S)[:, None] + np.arange(V)[None, :]
    mask = idx < S
    si, ji = np.nonzero(mask)
    padded = np.pad(x, ((0, 0), (0, V - 1), (0, 0)))
    visual = padded[:, idx, :]
    denom = np.arange(1, V + 1, dtype=np.float32)[None, None, :, None]
    context = np.cumsum(visual, axis=2, dtype=np.float32) / denom
    W1, W2, W3 = Wn[:, :H], Wn[:, H:2 * H], Wn[:, 2 * H:]
    rep = x @ W1.T
    vis = (visual.reshape(-1, H) @ W2.T).reshape(B, S, V, H)
    ctx = (context.reshape(-1, H) @ W3.T).reshape(B, S, V, H)
    out = np.tanh(rep[:, :, None, :] + vis + ctx + bn)
    return np.ascontiguousarray(out[:, si, ji, :].astype(np.float32))


def kernel(seq_hiddens, W, b):
    try:
        out, _ = run_hw(seq_hiddens, W, b, trace=False)
        return out
    except Exception:
        return _compute_np(seq_hiddens, W, b)
